# revision 29
# baseline (speedup 1.0000x reference)
"""Trainium2 Bass kernel v3 for nn_BLHmmLm (HMM language model evidence).

Same rank-D collapse as v2 (see kernel_v2_baseline.py docstring):
  trans[i,j] = Ex[i].Ey[j]/Z_i, emission[i,x] = Et[i].Ev[x]/den_i,
  w_t = M_t w_{t-1} with M_t = sum_e g_t[e] G[e,:,:],
  G[e,a,b] = sum_j Et[j,e] ExZd[j,a] Ey[j,b].

v3 changes (all perf):
- Single activation-table set: rsqrt computed as exp(-0.5*ln(s)) so every
  scalar-engine op (Exp/Ln/Relu/Copy/Square) lives in one act-func set ->
  one LoadActFuncSet instead of ~128 (~1.3us each).
- Host ships transposed layouts ([H, rows]) so the factor matmuls
  lhsT=proj need no per-tile PE transposes of input rows.
- Vocab Ev computed once into a DRAM table; the 4096 token rows are
  indirect-gathered from it instead of recomputed (drops 32 more
  normalize+project+exp tiles).
- Token rows go through a bf16 ReduceScatter (131KB out) instead of a
  2.1MB f32 AllReduce; LAM is folded into the ownership mask so the RS
  output is directly the LAM-scaled g used by the chain/M-build.
  wy/wv/Q ride a tiny [128,3] f32 AllReduce overlapped with the MLP.
- G AllReduce is split into 4 chunks (a-dim) pipelined against the
  G-build (producer) and the forward M-build (consumer).
- The T-step sequential chain is split into forward (w_t) and backward
  (r^T) halves that meet in the middle: 127 dependent steps instead of
  254. The backward half applies M^T, built from the same G via strided
  b-slices.
"""

import os
import sys
from contextlib import ExitStack

import numpy as np
import ml_dtypes

BF = np.dtype(ml_dtypes.bfloat16)

for _p in ("/opt/trn_rl_repo", "/root/.axon_site/_ro/trn_rl_repo"):
    if _p not in sys.path:
        sys.path.insert(0, _p)

import concourse.bass as bass
import concourse.bacc as bacc_mod
import concourse.tile as tile
from concourse import mybir
from concourse.bass_utils import run_bass_kernel_spmd
from concourse.masks import make_identity

F32 = mybir.dt.float32
BF16 = mybir.dt.bfloat16
I32 = mybir.dt.int32
AF = mybir.ActivationFunctionType
ALU = mybir.AluOpType
AX = mybir.AxisListType

C, V, H, D, N, T = 4096, 32000, 256, 128, 16, 256
NCORES = 8
CS = C // NCORES          # 512 states / core
VS = V // NCORES          # 4000 vocab rows / core
VSP = 4096                # padded V shard
NS = N // NCORES          # 2 sequences / core
P = 128
CT = CS // P              # 4 state tiles per shard
NTOK = N * T              # 4096 token instances
LTOK = NS * T             # 512 token instances per core
EPS = 1e-30
LAM = 32500.0             # ~exp(10.39): per-step rescale; only affects range
LNLAM = float(np.log(LAM))
NOCC = os.environ.get("KNOCC", "") != ""

AG_SZ = 4 * 32 * H        # tw shard floats per rank

HSTEP = 127               # chain steps per direction (fwd t=1..127,
                          # bwd t=254..128), meeting after t=127
MTOK = NS * HSTEP         # 254 M matrices per direction per core

NCHUNK = 4                # AR2 chunks (2 g8 iterations each)
CH_COLS = 8 * 512         # G cols per chunk
CH0_COLS = CH_COLS + 2 * P  # chunk0 also carries HT, KT


def _constrained_act_table_loads(self):
    """Restrict the act-table chooser to the one set that holds every
    function this kernel uses (Exp/Ln/Relu/Copy/Square/Identity), so the
    whole kernel needs a single LoadActFuncSet instead of reloading on
    every Ln<->Exp alternation.  Set ids stay positionally aligned with
    act_info.json (other entries are passed as empty, not removed)."""
    import bass_rust as _bass_rust
    from concourse.hw_specs import get_activation_tables

    has_activation = any(
        isinstance(i, mybir.InstActivation)
        for b in self.main_func.blocks
        for i in b.instructions
    )
    if not has_activation:
        return
    tabs = list(get_activation_tables(self.m.arch).items())
    keep = "natural_log_exp_and_others"
    assert any(nm == keep for nm, _ in tabs)
    tabs = [(nm, (s if nm == keep else set())) for nm, s in tabs]
    _bass_rust.insert_act_table_loads(self, tabs)


def _build_nc(reps=1):
    """reps>1 repeats the whole body (for amortised timing in test.py)."""
    import types
    nc = bacc_mod.Bacc()
    nc.insert_act_table_loads = types.MethodType(
        _constrained_act_table_loads, nc)
    ins = {}
    for nm, shp, dt in [
        ("stateT", [P, 2, CS], BF16), ("nextT", [P, 2, CS], BF16),
        ("pretT", [P, 2, CS], BF16), ("termT", [P, 2, VSP], BF16),
        ("proj", [H, D], F32), ("fxcol", [P, 2], F32),
        ("twsh", [4, 32, H], F32), ("tbf", [4, H], F32),
        ("gidx", [NTOK], I32), ("ownm", [NTOK], F32),
    ]:
        ins[nm] = nc.declare_dram_parameter(nm, shp, dt, isOutput=False)

    evid_out = nc.declare_dram_parameter("evid", [1, NS], F32, isOutput=True)
    groups = [list(range(NCORES))]

    with tile.TileContext(nc) as tc:
        for rep in range(reps):
            _emit_rep(nc, tc, ins, evid_out, groups, rep)
    return nc


def _emit_rep(nc, tc, ins, evid_out, groups, rep):
    sfx = "_%d" % rep
    ag_in = nc.dram_tensor("ag_in" + sfx, [AG_SZ], F32)
    ag_out = nc.dram_tensor("ag_out" + sfx, [NCORES, AG_SZ], F32,
                            addr_space="Shared")
    evtab = nc.dram_tensor("evtab" + sfx, [VSP, D], BF16)
    rs_in = nc.dram_tensor("rs_in" + sfx, [NTOK, D], BF16)
    rs_out = nc.dram_tensor("rs_out" + sfx, [LTOK, D], BF16)
    ars_in = nc.dram_tensor("ars_in" + sfx, [P, 3], F32)
    ars_out = nc.dram_tensor("ars_out" + sfx, [P, 3], F32,
                             addr_space="Shared")
    ar2_in = []
    ar2_out = []
    for c in range(NCHUNK):
        cols = CH0_COLS if c == 0 else CH_COLS
        ar2_in.append(nc.dram_tensor("ar2i%d%s" % (c, sfx), [P, cols], BF16))
        ar2_out.append(nc.dram_tensor("ar2o%d%s" % (c, sfx), [P, cols], BF16,
                                      addr_space="Shared"))

    def coll(kind, op, i, o):
        if not NOCC:
            nc.gpsimd.collective_compute(kind, op, replica_groups=groups,
                                         ins=[i], outs=[o])
        else:
            if kind == "AllGather":
                for _r in range(NCORES):
                    nc.sync.dma_start(out=o.tensor[_r, :], in_=i)
            elif kind == "ReduceScatter":
                nc.sync.dma_start(out=o, in_=i.tensor[0:LTOK, :])
            else:
                nc.sync.dma_start(out=o, in_=i)

    with ExitStack() as ctx:
        consts = ctx.enter_context(tc.tile_pool(name="consts" + sfx, bufs=1))
        big = ctx.enter_context(tc.tile_pool(name="big" + sfx, bufs=1))

        ident = consts.tile([P, P], F32)
        make_identity(nc, ident[:])
        ones_col = consts.tile([P, 1], F32)
        nc.vector.memset(ones_col[:], 1.0)
        ones_colb = consts.tile([P, 1], BF16)
        nc.vector.memset(ones_colb[:], 1.0)
        ones_row = consts.tile([1, P], F32)
        nc.vector.memset(ones_row[:], 1.0)
        ones_rowb = consts.tile([1, P], BF16)
        nc.vector.memset(ones_rowb[:], 1.0)
        zero_col = consts.tile([P, 1], F32)
        nc.vector.memset(zero_col[:], 0.0)
        eps_col = consts.tile([P, 1], F32)
        nc.vector.memset(eps_col[:], EPS)
        nh_col = consts.tile([P, 1], F32)
        nc.vector.memset(nh_col[:], -0.5)

        # persistent tiles (MbF/MbT live in a later pool, after setup frees)
        G3 = big.tile([P, P, P], BF16)         # [e, a, b]
        HTs = big.tile([P, P], BF16)           # [e, b]
        KTs = big.tile([P, P], BF16)           # [e, a]
        EvG = big.tile([P, NS, T], BF16)       # [e, n, t] LAM-scaled g
        Et_bf = big.tile([P, CT, D], BF16)     # [j, e] rows
        Ey_bf = big.tile([P, CT, D], BF16)     # [j, b] rows
        exzd = big.tile([P, CT, D], F32)       # [j, a] = Ex/(Z*den) rows
        Qt = big.tile([1, 1], F32)
        accln = big.tile([1, 4], F32)
        nc.vector.memset(accln[:], 0.0)

        # ---- small loads ----
        proj_sb = consts.tile([P, 2, D], F32)
        nc.sync.dma_start(out=proj_sb[:],
                          in_=ins["proj"].rearrange("(c p) o -> p c o", p=P))
        proj_bf = consts.tile([P, 2, D], BF16)
        nc.vector.tensor_copy(proj_bf[:], proj_sb[:])
        fx_sb = consts.tile([P, 2], F32)
        nc.sync.dma_start(out=fx_sb[:], in_=ins["fxcol"][:])
        gidx_sb = consts.tile([P, NTOK // P], I32)
        nc.sync.dma_start(out=gidx_sb[:],
                          in_=ins["gidx"].rearrange("(c p) -> p c", p=P))
        own_sb = consts.tile([P, NTOK // P], F32)
        nc.sync.dma_start(out=own_sb[:],
                          in_=ins["ownm"].rearrange("(c p) -> p c", p=P))
        tb_sb = consts.tile([P, 4, 2], F32)
        nc.sync.dma_start(out=tb_sb[:],
                          in_=ins["tbf"].rearrange("w (c p) -> p w c", p=P))

        # tw AllGather launched ASAP (overlaps the whole setup)
        nc.sync.dma_start(out=ag_in[:],
                          in_=ins["twsh"].rearrange("w r f -> (w r f)"))
        coll("AllGather", ALU.bypass, ag_in[:], ag_out[:])

        with tc.tile_pool(name="sp" + sfx, bufs=3) as sp, \
             tc.tile_pool(name="sp1" + sfx, bufs=1) as sp1, \
             tc.tile_pool(name="pss" + sfx, bufs=3, space="PSUM") as pss, \
             tc.tile_pool(name="pss2" + sfx, bufs=2, space="PSUM") as pss2:

            def ps_tile():
                return pss.tile([P, 512], F32, tag="ps", name="pst")

            def pe_transpose(in_ap, pp, ff, pool=None, tag="tr", dt=F32,
                             eng=None):
                """in_ap [pp, ff] -> sbuf tile [ff, pp]."""
                ps = ps_tile()[:ff, :pp]
                nc.tensor.transpose(ps, in_ap, ident[:pp, :pp])
                out = (pool or sp).tile([ff, pp], dt, tag=tag)
                if eng is nc.scalar:
                    nc.scalar.copy(out[:], ps)
                else:
                    (eng or nc.vector).tensor_copy(out[:], ps)
                return out

            def rsqrt_row(dst, src_ps, n):
                """dst[1,n] = 1/sqrt(src_ps[1,n]) via exp(-0.5*ln(x))."""
                lnr = sp.tile([1, n], F32, tag="lnr")
                nc.scalar.activation(lnr[:], src_ps, AF.Ln,
                                     bias=eps_col[:1])
                nc.scalar.activation(dst, lnr[:], AF.Exp, scale=-0.5)

            def bcast_row(row_ap, n, tag="bc"):
                """[1,n] row -> [P,n] bf16 tile (every partition = row)."""
                ps = ps_tile()[:, :n]
                nc.tensor.matmul(ps, ones_row[:], row_ap, start=True,
                                 stop=True)
                out = sp.tile([P, n], BF16, tag=tag)
                nc.vector.tensor_copy(out[:], ps)
                return out

            # ============ vocab loop: EvT tiles + wv + evtab ============
            NVT = VSP // 512                       # 8 tiles of 512 cols
            wv8 = sp1.tile([P, NVT], F32, tag="wv8")
            with tc.tile_pool(name="vt" + sfx, bufs=3) as vt, \
                 tc.tile_pool(name="vps" + sfx, bufs=3,
                              space="PSUM") as vps:
                for v8 in range(NVT):
                    tcol = vt.tile([P, 2, 512], BF16, tag="tcol")
                    nc.sync.dma_start(
                        out=tcol[:],
                        in_=ins["termT"][:, :, v8 * 512:(v8 + 1) * 512])
                    sq = vt.tile([P, 2, 512], BF16, tag="sq")
                    nc.scalar.activation(sq[:, 0, :], tcol[:, 0, :],
                                         AF.Square)
                    nc.vector.tensor_mul(sq[:, 1, :], tcol[:, 1, :],
                                         tcol[:, 1, :])
                    psn = vps.tile([P, 512], F32, tag="vps",
                                   name="vn")[:1, :]
                    for hh in range(2):
                        nc.tensor.matmul(psn, ones_colb[:], sq[:, hh, :],
                                         start=(hh == 0), stop=(hh == 1))
                    invn = sp.tile([1, 512], F32, tag="invn")
                    rsqrt_row(invn[:], psn, 512)
                    invbc = bcast_row(invn[:], 512)
                    tsc = vt.tile([P, 2, 512], BF16, tag="tsc")
                    nc.vector.tensor_mul(tsc[:, 0, :], tcol[:, 0, :],
                                         invbc[:])
                    nc.vector.tensor_mul(tsc[:, 1, :], tcol[:, 1, :],
                                         invbc[:])
                    psx = vps.tile([P, 512], F32, tag="vps", name="vx")
                    for hh in range(2):
                        nc.tensor.matmul(psx[:], proj_bf[:, hh, :],
                                         tsc[:, hh, :],
                                         start=(hh == 0), stop=(hh == 1))
                    evt = vt.tile([P, 512], F32, tag="evt")
                    nc.scalar.activation(evt[:], psx[:], AF.Exp,
                                         bias=nh_col[:])
                    # wv partial (mask pad cols of the last tile)
                    ncol = 512 if v8 < NVT - 1 else (VS - (NVT - 1) * 512)
                    nc.vector.tensor_reduce(wv8[:, v8:v8 + 1],
                                            evt[:, :ncol], axis=AX.X,
                                            op=ALU.add)
                    # transpose to rows and store to evtab
                    for cc in range(4):
                        rowst = pe_transpose(evt[:, cc * P:(cc + 1) * P],
                                             P, P, pool=vt, tag="evr",
                                             dt=BF16,
                                             eng=(nc.vector if cc % 2 == 0
                                                  else nc.scalar))
                        nc.sync.dma_start(
                            out=evtab[v8 * 512 + cc * P:
                                      v8 * 512 + (cc + 1) * P, :],
                            in_=rowst[:])
            wv_col = sp1.tile([P, 1], F32, tag="wvc")
            nc.vector.tensor_reduce(wv_col[:], wv8[:], axis=AX.X, op=ALU.add)

            # ============ token gather -> rs_in, ReduceScatter ============
            with tc.tile_pool(name="gat" + sfx, bufs=3) as gp:
                for gt in range(NTOK // P):
                    grows = gp.tile([P, D], BF16, tag="grow")
                    nc.gpsimd.indirect_dma_start(
                        out=grows[:], out_offset=None,
                        in_=evtab[:, :],
                        in_offset=bass.IndirectOffsetOnAxis(
                            ap=gidx_sb[:, gt:gt + 1], axis=0))
                    gm = gp.tile([P, D], BF16, tag="gm")
                    if gt % 2 == 0:
                        nc.vector.tensor_scalar_mul(gm[:], grows[:],
                                                    own_sb[:, gt:gt + 1])
                    else:
                        nc.scalar.mul(gm[:], grows[:], own_sb[:, gt:gt + 1])
                    nc.sync.dma_start(out=rs_in[gt * P:(gt + 1) * P, :],
                                      in_=gm[:])
            coll("ReduceScatter", ALU.add, rs_in[:], rs_out[:])

            # ============ ex0 from host-computed start-MLP fx ============
            sqf = sp.tile([P, 2], F32, tag="fxsq")
            nc.vector.tensor_mul(sqf[:], fx_sb[:], fx_sb[:])
            ssq = sp.tile([P, 1], F32, tag="fxss")
            nc.vector.tensor_reduce(ssq[:], sqf[:], axis=AX.X, op=ALU.add)
            psn0 = ps_tile()[:1, :1]
            nc.tensor.matmul(psn0, ones_col[:], ssq[:], start=True, stop=True)
            invfx = sp1.tile([1, 1], F32, tag="invfx")
            rsqrt_row(invfx[:], psn0, 1)
            ps0 = ps_tile()[:1, :D]
            for ic in range(2):
                nc.tensor.matmul(ps0, fx_sb[:, ic:ic + 1], proj_sb[:, ic, :],
                                 start=(ic == 0), stop=(ic == 1))
            ex0_row = sp.tile([1, D], F32, tag="ex0r")
            nc.scalar.activation(ex0_row[:], ps0, AF.Exp,
                                 bias=nh_col[:1], scale=invfx[:])
            ex0_col = pe_transpose(ex0_row[:], 1, D, pool=sp1, tag="ex0c")

            # ============ state factors in transposed layout ============
            def state_factorT(dram, tagp):
                """dram [P,2,CS] (transposed rows) -> [d, j] factor tile."""
                rT = sp.tile([P, 2, CS], BF16, tag="rT")
                nc.sync.dma_start(out=rT[:], in_=dram[:])
                sq = sp.tile([P, 2, CS], BF16, tag="rsq")
                nc.scalar.activation(sq[:, 0, :], rT[:, 0, :], AF.Square)
                nc.vector.tensor_mul(sq[:, 1, :], rT[:, 1, :], rT[:, 1, :])
                psn = pss2.tile([P, 512], F32, tag="ps2",
                                name="sn" + tagp)[:1, :]
                for hh in range(2):
                    nc.tensor.matmul(psn, ones_colb[:], sq[:, hh, :],
                                     start=(hh == 0), stop=(hh == 1))
                invn = sp.tile([1, CS], F32, tag="sinv")
                rsqrt_row(invn[:], psn, CS)
                invbc = bcast_row(invn[:], CS)
                tsc = sp.tile([P, 2, CS], BF16, tag="stsc")
                nc.vector.tensor_mul(tsc[:, 0, :], rT[:, 0, :], invbc[:])
                nc.vector.tensor_mul(tsc[:, 1, :], rT[:, 1, :], invbc[:])
                psx = pss2.tile([P, 512], F32, tag="ps2", name="sx" + tagp)
                for hh in range(2):
                    nc.tensor.matmul(psx[:], proj_bf[:, hh, :], tsc[:, hh, :],
                                     start=(hh == 0), stop=(hh == 1))
                fac = sp1.tile([P, CS], F32, tag=tagp + "T")
                nc.scalar.activation(fac[:], psx[:], AF.Exp, bias=nh_col[:])
                return fac

            ExT = state_factorT(ins["stateT"], "ex")      # [d, j]
            EyT = state_factorT(ins["nextT"], "ey")       # [d, j]

            # wy partial: row-sum of EyT -> column [d, 1]
            wy_col = sp1.tile([P, 1], F32, tag="wyc")
            nc.vector.tensor_reduce(wy_col[:], EyT[:], axis=AX.X, op=ALU.add)
            # q row [1, CS] = ex0 . Ey[j]
            psq = ps_tile()[:1, :CS]
            nc.tensor.matmul(psq, ex0_col[:], EyT[:], start=True, stop=True)
            q_row = sp1.tile([1, CS], F32, tag="qrow")
            nc.vector.tensor_copy(q_row[:], psq)
            qred = sp.tile([1, 1], F32, tag="qred")
            nc.vector.tensor_reduce(qred[:], q_row[:], axis=AX.X, op=ALU.add)

            # ---- tiny AllReduce: [wy | wv | Q] ----
            nc.sync.dma_start(out=ars_in[:, 0:1], in_=wy_col[:])
            nc.sync.dma_start(out=ars_in[:, 1:2], in_=wv_col[:])
            nc.sync.dma_start(out=ars_in[:, 2:3], in_=zero_col[:])
            nc.sync.dma_start(out=ars_in[0:1, 2:3], in_=qred[:])
            coll("AllReduce", ALU.add, ars_in[:], ars_out[:])

            # ============ terminal MLP (transposed) ============
            w_all = sp1.tile([P, 4, 2, H], F32, tag="twall")
            for k in range(NCORES):
                p0 = (k % 4) * 32
                nc.sync.dma_start(
                    out=w_all[p0:p0 + 32, :, k // 4, :],
                    in_=ag_out[k].rearrange("(w r f) -> r w f", w=4, r=32))
            pT = sp1.tile([P, 2, CS], F32, tag="pT")
            pTh = sp.tile([P, 2, CS], BF16, tag="rT")
            nc.sync.dma_start(out=pTh[:], in_=ins["pretT"][:])
            nc.vector.tensor_copy(pT[:], pTh[:])

            def lin_big(srcT, wi):
                dst = sp1.tile([P, 2, CS], F32, tag="mlph%d" % (wi % 2))
                for oc in range(2):
                    ps = pss2.tile([P, 512], F32, tag="ps2")
                    for ic in range(2):
                        nc.tensor.matmul(
                            ps, w_all[:, wi, ic, oc * P:(oc + 1) * P],
                            srcT[:, ic, :],
                            start=(ic == 0), stop=(ic == 1))
                    nc.scalar.activation(dst[:, oc, :], ps, AF.Relu,
                                         bias=tb_sb[:, wi, oc:oc + 1])
                return dst

            h = lin_big(pT, 0)
            h = lin_big(h, 1)
            ft1 = sp1.tile([P, 2, CS], F32, tag="ft1")
            nc.vector.tensor_add(ft1[:], h[:], pT[:])
            h = lin_big(ft1, 2)
            h = lin_big(h, 3)
            ftT = sp1.tile([P, 2, CS], F32, tag="ftT")
            nc.vector.tensor_add(ftT[:], h[:], ft1[:])
            sqT = sp.tile([P, 2, CS], F32, tag="sqT")
            nc.vector.tensor_mul(sqT[:, 0, :], ftT[:, 0, :], ftT[:, 0, :])
            nc.scalar.activation(sqT[:, 1, :], ftT[:, 1, :], AF.Square)
            psf = pss2.tile([P, 512], F32, tag="ps2", name="ftn")[:1, :]
            for ic in range(2):
                nc.tensor.matmul(psf, ones_col[:], sqT[:, ic, :],
                                 start=(ic == 0), stop=(ic == 1))
            invft = sp.tile([1, CS], F32, tag="invft")
            rsqrt_row(invft[:], psf, CS)
            invfb = bcast_row(invft[:], CS, tag="bcf")
            fts = sp.tile([P, 2, CS], BF16, tag="fts")
            nc.vector.tensor_mul(fts[:, 0, :], ftT[:, 0, :], invfb[:])
            nc.vector.tensor_mul(fts[:, 1, :], ftT[:, 1, :], invfb[:])
            psx = pss2.tile([P, 512], F32, tag="ps2", name="etx")
            for hh in range(2):
                nc.tensor.matmul(psx[:], proj_bf[:, hh, :], fts[:, hh, :],
                                 start=(hh == 0), stop=(hh == 1))
            EtT = sp1.tile([P, CS], F32, tag="EtT")
            nc.scalar.activation(EtT[:], psx[:], AF.Exp, bias=nh_col[:])

            # ============ post-ars: Z, den, izd, invden ============
            wy2 = sp.tile([P, 1], F32, tag="wy2")
            nc.sync.dma_start(out=wy2[:], in_=ars_out[:, 0:1])
            wv2 = sp.tile([P, 1], F32, tag="wv2")
            nc.sync.dma_start(out=wv2[:], in_=ars_out[:, 1:2])
            nc.sync.dma_start(out=Qt[:], in_=ars_out[0:1, 2:3])

            psZ = pss2.tile([P, 512], F32, tag="ps2", name="psZ")[:1, :CS]
            nc.tensor.matmul(psZ, wy2[:], ExT[:], start=True, stop=True)
            psDen = pss2.tile([P, 512], F32, tag="ps2", name="psDen")[:1, :CS]
            nc.tensor.matmul(psDen, wv2[:], EtT[:], start=True, stop=True)
            den_row = sp.tile([1, CS], F32, tag="denr")
            nc.vector.tensor_copy(den_row[:], psDen)
            izd_row = sp1.tile([1, CS], F32, tag="izdr")
            nc.vector.tensor_mul(izd_row[:], psZ, den_row[:])
            nc.vector.reciprocal(izd_row[:], izd_row[:])
            invd_row = sp1.tile([1, CS], F32, tag="invdr")
            nc.vector.reciprocal(invd_row[:], den_row[:])

            # transpose small rows to per-partition scalar columns
            izd4 = sp1.tile([P, CT], F32, tag="izd4")
            invd4 = sp1.tile([P, CT], F32, tag="invd4")
            q4 = sp1.tile([P, CT], F32, tag="q4")
            for st in range(CT):
                t = pe_transpose(izd_row[:, st * P:(st + 1) * P], 1, P)
                nc.vector.tensor_copy(izd4[:, st:st + 1], t[:])
                t = pe_transpose(invd_row[:, st * P:(st + 1) * P], 1, P)
                nc.vector.tensor_copy(invd4[:, st:st + 1], t[:])
                t = pe_transpose(q_row[:, st * P:(st + 1) * P], 1, P)
                nc.vector.tensor_copy(q4[:, st:st + 1], t[:])

            # rows layouts: Ey, Ex(->exzd), Et  (transpose factors back)
            Ey_f = sp1.tile([P, CT, D], F32, tag="eyrows")
            for st in range(CT):
                t = pe_transpose(EyT[:, st * P:(st + 1) * P], P, P,
                                 eng=(nc.vector if st % 2 == 0
                                      else nc.scalar))
                nc.vector.tensor_copy(Ey_f[:, st, :], t[:])
            nc.scalar.copy(Ey_bf[:], Ey_f[:])
            for st in range(CT):
                t = pe_transpose(ExT[:, st * P:(st + 1) * P], P, P,
                                 eng=(nc.vector if st % 2 == 0
                                      else nc.scalar))
                nc.vector.tensor_scalar_mul(exzd[:, st, :], t[:],
                                            izd4[:, st:st + 1])
                t = pe_transpose(EtT[:, st * P:(st + 1) * P], P, P,
                                 eng=(nc.scalar if st % 2 == 0
                                      else nc.vector))
                nc.scalar.copy(Et_bf[:, st, :], t[:])

            eyden = sp.tile([P, CT, D], BF16, tag="eyden")
            exq = sp.tile([P, CT, D], BF16, tag="exq")
            for st in range(CT):
                eng = nc.vector if st % 2 == 0 else nc.scalar
                if st % 2 == 0:
                    nc.vector.tensor_scalar_mul(eyden[:, st, :],
                                                Ey_f[:, st, :],
                                                invd4[:, st:st + 1])
                    nc.scalar.mul(exq[:, st, :], exzd[:, st, :],
                                  q4[:, st:st + 1])
                else:
                    nc.scalar.mul(eyden[:, st, :], Ey_f[:, st, :],
                                  invd4[:, st:st + 1])
                    nc.vector.tensor_scalar_mul(exq[:, st, :],
                                                exzd[:, st, :],
                                                q4[:, st:st + 1])

            # HT/KT partials -> chunk0 tail cols
            ps_ht = pss2.tile([P, 512], F32, tag="ps2", name="ps_ht")[:, :D]
            for st in range(CT):
                nc.tensor.matmul(ps_ht, Et_bf[:, st, :], eyden[:, st, :],
                                 start=(st == 0), stop=(st == CT - 1))
            ht_part = sp.tile([P, D], BF16, tag="htp")
            nc.vector.tensor_copy(ht_part[:], ps_ht)
            nc.sync.dma_start(out=ar2_in[0][:, CH_COLS:CH_COLS + P],
                              in_=ht_part[:])
            ps_kt = pss2.tile([P, 512], F32, tag="ps2", name="ps_kt")[:, :D]
            for st in range(CT):
                nc.tensor.matmul(ps_kt, Et_bf[:, st, :], exq[:, st, :],
                                 start=(st == 0), stop=(st == CT - 1))
            kt_part = sp.tile([P, D], BF16, tag="ktp")
            nc.vector.tensor_copy(kt_part[:], ps_kt)
            nc.sync.dma_start(out=ar2_in[0][:, CH_COLS + P:CH_COLS + 2 * P],
                              in_=kt_part[:])

            # EvG build from rs_out via XBAR DMA transpose (LAM already
            # folded into the ownership mask): [T, D] rows -> [D, T] cols
            for n in range(NS):
                nc.sync.dma_start(out=EvG[:, n, :],
                                  in_=rs_out[n * T:(n + 1) * T, :],
                                  transpose=True)

        # ---- G partial build + chunked AllReduce ----
        with tc.tile_pool(name="gtp" + sfx, bufs=4) as gtp, \
             tc.tile_pool(name="gcp" + sfx, bufs=2) as gcp, \
             tc.tile_pool(name="gps" + sfx, bufs=4, space="PSUM") as gps:
            for g8 in range(8):
                psG = [gps.tile([P, 512], F32, tag="psg", name="psg%d" % c4)
                       for c4 in range(4)]
                for jt in range(CT):
                    for c4 in range(4):
                        c = g8 * 4 + c4
                        a0 = 4 * c
                        tmp = gtp.tile([P, 512], BF16, tag="tmp")
                        for ai in range(4):
                            if ai == 3:
                                nc.scalar.mul(
                                    tmp[:, ai * P:(ai + 1) * P],
                                    Ey_bf[:, jt, :],
                                    exzd[:, jt, a0 + ai:a0 + ai + 1])
                            else:
                                nc.vector.tensor_scalar_mul(
                                    tmp[:, ai * P:(ai + 1) * P],
                                    Ey_bf[:, jt, :],
                                    exzd[:, jt, a0 + ai:a0 + ai + 1])
                        nc.tensor.matmul(psG[c4][:], Et_bf[:, jt, :], tmp[:],
                                         start=(jt == 0), stop=(jt == CT - 1))
                chunk = g8 // 2
                for c4 in range(4):
                    c = g8 * 4 + c4
                    c_loc = c - chunk * 8
                    gsl = gcp.tile([P, 512], BF16, tag="gsl")
                    nc.scalar.copy(gsl[:], psG[c4][:])
                    nc.sync.dma_start(
                        out=ar2_in[chunk][:, c_loc * 512:(c_loc + 1) * 512],
                        in_=gsl[:])
                if g8 % 2 == 1:
                    coll("AllReduce", ALU.add, ar2_in[chunk][:],
                         ar2_out[chunk][:])

        # ---- M builds (fwd per chunk; bwd after all chunks) ----
        mbp = ctx.enter_context(tc.tile_pool(name="mbp" + sfx, bufs=1))
        MbF = mbp.tile([P, MTOK, P], BF16)     # [b, (n,t-1), a] = M_t[a,b]
        MbT = mbp.tile([P, MTOK, P], BF16)     # [a, (n,t-128), b] = M_t[a,b]
        with tc.tile_pool(name="mps" + sfx, bufs=4, space="PSUM") as mps:
            for chunk in range(NCHUNK):
                nc.sync.dma_start(
                    out=G3[:, chunk * 32:(chunk + 1) * 32, :],
                    in_=ar2_out[chunk][:, 0:CH_COLS])
                if chunk == 0:
                    nc.sync.dma_start(out=HTs[:],
                                      in_=ar2_out[0][:, CH_COLS:
                                                     CH_COLS + P])
                    nc.sync.dma_start(out=KTs[:],
                                      in_=ar2_out[0][:, CH_COLS + P:
                                                     CH_COLS + 2 * P])
                for ar in range(32):
                    a = chunk * 32 + ar
                    psA = mps.tile([P, MTOK], F32, tag="psA")
                    nc.tensor.matmul(psA[:], G3[:, a, :],
                                     EvG[:, :, 1:1 + HSTEP], start=True,
                                     stop=True)
                    if a % 2 == 0:
                        nc.vector.tensor_copy(MbF[:, :, a], psA[:])
                    else:
                        nc.scalar.copy(MbF[:, :, a], psA[:])
            for b in range(P):
                psB = mps.tile([P, MTOK], F32, tag="psA")
                nc.tensor.matmul(psB[:], G3[:, :, b],
                                 EvG[:, :, P:P + HSTEP], start=True,
                                 stop=True)
                if b % 2 == 0:
                    nc.vector.tensor_copy(MbT[:, :, b], psB[:])
                else:
                    nc.scalar.copy(MbT[:, :, b], psB[:])

        # ---- chains: fwd w (cols 0,1) + bwd r (cols 2,3) ----
        with tc.tile_pool(name="cp" + sfx, bufs=2) as cp, \
             tc.tile_pool(name="cp1" + sfx, bufs=1) as cp1, \
             tc.tile_pool(name="cps" + sfx, bufs=2, space="PSUM") as cps, \
             tc.tile_pool(name="rps" + sfx, bufs=2, space="PSUM") as rps:

            psW = cps.tile([P, 4], F32, tag="pw")
            for n in range(NS):
                nc.tensor.matmul(psW[:, n:n + 1], KTs[:], EvG[:, n, 0:1],
                                 start=True, stop=True)
                nc.tensor.matmul(psW[:, 2 + n:3 + n], HTs[:],
                                 EvG[:, n, T - 1:T], start=True, stop=True)
            w4 = cp.tile([P, 4], BF16, tag="w4")
            nc.vector.tensor_copy(w4[:], psW[:])

            for s in range(1, HSTEP + 1):
                psW = cps.tile([P, 4], F32, tag="pw")
                for n in range(NS):
                    cf = n * HSTEP + (s - 1)
                    nc.tensor.matmul(psW[:, n:n + 1], MbF[:, cf, :],
                                     w4[:, n:n + 1], start=True, stop=True)
                    cb = n * HSTEP + (HSTEP - s)
                    nc.tensor.matmul(psW[:, 2 + n:3 + n], MbT[:, cb, :],
                                     w4[:, 2 + n:3 + n], start=True,
                                     stop=True)
                w4n = cp.tile([P, 4], BF16, tag="w4")
                nc.vector.tensor_copy(w4n[:], psW[:])
                w4 = w4n
                if s == 64:
                    w4r = cp.tile([P, 4], BF16, tag="w4")
                    for j in range(4):
                        psS = rps.tile([1, 1], F32, tag="prn")
                        nc.tensor.matmul(psS[:], ones_colb[:],
                                         w4[:, j:j + 1], start=True,
                                         stop=True)
                        sS = cp.tile([1, 1], F32, tag="sS%d" % j)
                        nc.vector.tensor_copy(sS[:], psS[:])
                        lnS = cp.tile([1, 1], F32, tag="lnS%d" % j)
                        nc.scalar.activation(lnS[:], sS[:], AF.Ln)
                        nc.vector.tensor_add(accln[:, j:j + 1],
                                             accln[:, j:j + 1], lnS[:])
                        rcf = cp.tile([1, 1], F32, tag="rcf%d" % j)
                        nc.vector.reciprocal(rcf[:], sS[:])
                        rc = cp.tile([1, 1], BF16, tag="rc%d" % j)
                        nc.vector.tensor_copy(rc[:], rcf[:])
                        psB2 = rps.tile([P, 1], F32, tag="prn")
                        nc.tensor.matmul(psB2[:], ones_rowb[:], rc[:],
                                         start=True, stop=True)
                        bc = cp.tile([P, 1], F32, tag="bc%d" % j)
                        nc.vector.tensor_copy(bc[:], psB2[:])
                        nc.vector.tensor_mul(w4r[:, j:j + 1],
                                             w4[:, j:j + 1], bc[:])
                    w4 = w4r

            # ---- finale: ev_n = ln(r_n.w_n) + acc - (lnQ + T*lnLAM) ----
            lnq = cp1.tile([1, 1], F32, tag="lnq")
            nc.scalar.activation(lnq[:], Qt[:], AF.Ln)
            ev2 = cp1.tile([1, NS], F32, tag="ev2")
            for n in range(NS):
                wcol = cp.tile([P, 1], BF16, tag="wc%d" % n)
                nc.vector.tensor_copy(wcol[:], w4[:, n:n + 1])
                psD = rps.tile([1, 1], F32, tag="prn")
                nc.tensor.matmul(psD[:], wcol[:], w4[:, 2 + n:3 + n],
                                 start=True, stop=True)
                dsb = cp.tile([1, 1], F32, tag="dsb%d" % n)
                nc.vector.tensor_copy(dsb[:], psD[:])
                lnD = cp.tile([1, 1], F32, tag="lnD%d" % n)
                nc.scalar.activation(lnD[:], dsb[:], AF.Ln)
                nc.vector.tensor_add(ev2[:, n:n + 1], lnD[:],
                                     accln[:, n:n + 1])
                nc.vector.tensor_add(ev2[:, n:n + 1], ev2[:, n:n + 1],
                                     accln[:, 2 + n:3 + n])
            ofs = cp1.tile([1, 1], F32, tag="ofs")
            nc.vector.memset(ofs[:], T * LNLAM)
            lnqofs = cp1.tile([1, 1], F32, tag="lnqofs")
            nc.vector.tensor_add(lnqofs[:], lnq[:], ofs[:])
            lnq2 = cp1.tile([1, NS], F32, tag="lnq2")
            for n in range(NS):
                nc.vector.tensor_copy(lnq2[:, n:n + 1], lnqofs[:])
            nc.vector.tensor_tensor(out=ev2[:], in0=ev2[:], in1=lnq2[:],
                                    op=ALU.subtract)
            nc.sync.dma_start(out=evid_out[:], in_=ev2[:])


def _host_start_mlp(inputs):
    def f32(x):
        return np.asarray(x, dtype=np.float32)

    def res(x, w1, b1, w2, b2):
        h = np.maximum(x @ w1 + b1, 0.0)
        return np.maximum(h @ w2 + b2, 0.0) + x

    fx = f32(inputs["start_emb"]) @ f32(inputs["sw0"]) + f32(inputs["sb0"])
    fx = res(fx, f32(inputs["sw1"]), f32(inputs["sb1"]),
             f32(inputs["sw2"]), f32(inputs["sb2"]))
    fx = res(fx, f32(inputs["sw3"]), f32(inputs["sb3"]),
             f32(inputs["sw4"]), f32(inputs["sb4"]))
    return np.ascontiguousarray(fx.reshape(2, P).T)  # [128, 2]


def make_in_maps(inputs):
    text = np.asarray(inputs["text"])
    mask = np.asarray(inputs["mask"])
    assert bool(np.all(mask)), "kernel assumes mask is all ones"

    def f32(x):
        return np.ascontiguousarray(np.asarray(x), dtype=np.float32)

    def tshard(x, k, rows):
        """[rows_total, H] shard k -> transposed [P, 2, rows] bf16."""
        sh = np.asarray(x)[k * rows:(k + 1) * rows].astype(np.float32)
        t = sh.T.reshape(2, P, -1).transpose(1, 0, 2)  # [P, 2, rows]
        return np.ascontiguousarray(t).astype(BF)

    fxcol = _host_start_mlp(inputs)
    tw_all = np.stack([f32(inputs["tw%d" % i]) for i in (1, 2, 3, 4)])
    tb_all = np.stack([f32(inputs["tb%d" % i]) for i in (1, 2, 3, 4)])

    in_maps = []
    toks = text.reshape(NTOK).astype(np.int64)   # token c = n*T + t
    for k in range(NCORES):
        m = {}
        m["stateT"] = tshard(inputs["state_emb"], k, CS)
        m["nextT"] = tshard(inputs["next_state_emb"], k, CS)
        m["pretT"] = tshard(inputs["preterminal_emb"], k, CS)
        tsh = np.zeros((H, VSP), np.float32)
        tsh[:, :VS] = np.asarray(inputs["terminal_emb"]) \
            [k * VS:(k + 1) * VS].astype(np.float32).T
        m["termT"] = np.ascontiguousarray(
            tsh.reshape(2, P, VSP).transpose(1, 0, 2)).astype(BF)
        m["proj"] = f32(inputs["proj"])
        m["fxcol"] = fxcol
        m["twsh"] = np.ascontiguousarray(tw_all[:, k * 32:(k + 1) * 32, :])
        m["tbf"] = tb_all
        own = (toks >= k * VS) & (toks < (k + 1) * VS)
        m["gidx"] = np.where(own, toks - k * VS, 0).astype(np.int32)
        m["ownm"] = own.astype(np.float32) * np.float32(LAM)
        in_maps.append(m)
    return in_maps


_NC_CACHE = None


def kernel(**inputs):
    global _NC_CACHE
    if _NC_CACHE is None:
        _NC_CACHE = _build_nc()
        _NC_CACHE.finalize()
    res = run_bass_kernel_spmd(_NC_CACHE, make_in_maps(inputs),
                               list(range(NCORES)))
    ev = np.float32(0.0)
    for k in range(NCORES):
        ev += res.results[k]["evid"].reshape(NS).sum(dtype=np.float32)
    return np.float32(ev)


if __name__ == "__main__":
    dat = np.load("/root/problem/inputs.npz")
    out = kernel(**{k: dat[k] for k in dat.files})
    print("kernel evidence:", out)
    exp = np.load("/root/problem/expected.npy")
    rel = abs(float(out) - float(exp)) / max(abs(float(exp)), 1e-30)
    print("expected:", exp, " rel err: %.3e" % rel)
